# revision 2
# baseline (speedup 1.0000x reference)
"""Multi-head causal attention (B=4, S=2048, D=1024, H=16) on 8 NeuronCores.

Sharding: core c -> batch b = c//2, head-group g = c%2 (8 heads each).
Each core computes, for its batch and heads:
    QT/KT = W.T @ x.T          (transposed projections, [64, S] per head)
    V     = x @ Wv             (natural layout, plus ones column for denom)
    ST    = K_chunk @ Q_blk.T  ([k=128, q=512] score chunks, causal-skipped)
    E     = exp(ST/8) (* triangle mask on the partial diagonal block)
    accT  = V_aug.T @ E        ([65, q]: rows 0-63 unnormalized out.T, row 64 denom)
    out.T = accT[:64] / accT[64]  (stacked over heads -> concatT [512, S])
    y_part = concatT.T @ W_O_part.T
Host sums the two partial y's per batch (the "all-reduce after W_O").

v2 notes vs the original:
  - Single x stream: Q, K, V projections all computed from one pass over the
    chunks, as 12 per-chunk PSUM accumulation tasks (fits 8 banks since only
    bufs tasks are live at once).  Weight DMAs ride the ACT HWDGE queue so
    they don't queue behind x-chunk loads on the SP queue.
  - Attention packs the two heads of a pair into adjacent score matmuls on
    PE row groups 0-63 / 64-127 (concurrent in the array) writing separate
    PSUM banks, with one exp covering both heads' scores.
All matmuls run as float32r (full fp32 data, 1 cycle/row PE mode).
"""

import numpy as np

import concourse.bass as bass
import concourse.tile as tile
import concourse.mybir as mybir
from concourse import bacc
from concourse.bass_utils import run_bass_kernel_spmd

B, S, D, H, HD = 4, 2048, 1024, 16, 64
NH = 8            # heads per core
NP = NH // 2      # head pairs per core
QB = 512          # q block size
NQB = S // QB     # 4
KC = 128          # k chunk size
ND = QB // KC     # diagonal chunks per q block
NKT = D // 128    # 8 contraction tiles over D
NST = S // 128    # 16 s tiles
CW = NH * HD      # 512 concat width per core

F32 = mybir.dt.float32
F32R = mybir.dt.float32r
AF = mybir.ActivationFunctionType

N_CORES = 8

_cache = {}


def _r(ap):
    return ap.bitcast(F32R)


def build_nc(repeats=1, phases="full", hw_loop=False):
    nc = bacc.Bacc("TRN2", target_bir_lowering=False, debug=False,
                   num_devices=N_CORES)
    x_ck = nc.dram_tensor("x_ck", [NQB, 128, NKT, QB], F32R,
                          kind="ExternalInput").ap()
    wq = nc.dram_tensor("wq", [128, NKT, CW], F32R, kind="ExternalInput").ap()
    wk = nc.dram_tensor("wk", [128, NKT, CW], F32R, kind="ExternalInput").ap()
    wv = nc.dram_tensor("wv", [128, NKT, CW], F32R, kind="ExternalInput").ap()
    wot = nc.dram_tensor("wot", [128, CW // 128, D], F32R,
                         kind="ExternalInput").ap()
    masks = nc.dram_tensor("masks", [KC, KC], F32R, kind="ExternalInput").ap()
    ones = nc.dram_tensor("ones", [128, NST * NH], F32R,
                          kind="ExternalInput").ap()
    y = nc.dram_tensor("y", [S, D], F32, kind="ExternalOutput").ap()

    with tile.TileContext(nc) as tc:
        if hw_loop:
            with tc.For_i(0, repeats, 1):
                _build(tc, x_ck, wq, wk, wv, wot, masks, ones, y, phases)
        else:
            for _ in range(repeats):
                _build(tc, x_ck, wq, wk, wv, wot, masks, ones, y, phases)
    nc.compile()
    return nc


def _build(tc, x_ck, wq, wk, wv, wot, masks, ones, y, phases="full"):
    nc = tc.nc
    with tc.tile_pool(name="persist", bufs=1) as persist:
        qt_sb = persist.tile([128, NP, S], F32R)      # [2 heads, pair, s]
        kt_sb = persist.tile([128, NP, S], F32R)
        v_sb = persist.tile([128, NST, NH, HD + 1], F32R)
        tri_sb = persist.tile([128, KC], F32R)
        wot_sb = persist.tile([128, CW // 128, D], F32R)
        nc.sync.dma_start(tri_sb, masks)
        v_ones = bass.AP(tensor=v_sb.tensor, offset=v_sb.offset + HD,
                         ap=[list(v_sb.ap[0]), [HD + 1, NST * NH], [1, 1]])
        nc.sync.dma_start(v_ones, ones.rearrange("p (n o) -> p n o", o=1))

        # ---- projections: one streamed pass computes Q, K and V ----------
        run_proj = phases != "dma"
        with (
            tc.tile_pool(name="pj_w", bufs=1) as pjw,
            tc.tile_pool(name="pj_x", bufs=2) as pjx,
            tc.tile_pool(name="pj_p", bufs=1, space="PSUM") as pjp,
        ):
            wq_sb = pjw.tile([128, NKT, CW], F32R)
            wk_sb = pjw.tile([128, NKT, CW], F32R)
            wv_sb = pjw.tile([128, NKT, CW], F32R)
            # weight DMAs on the ACT HWDGE ring; x chunks ride the SP ring
            nc.scalar.dma_start(wq_sb, wq)
            nc.scalar.dma_start(wk_sb, wk)
            nc.scalar.dma_start(wv_sb, wv)
            nc.scalar.dma_start(wot_sb, wot)
            for c in range(NQB):
                csl = slice(c * QB, (c + 1) * QB)
                xs = pjx.tile([128, NKT, QB], F32R, tag="xs")
                nc.sync.dma_start(xs, x_ck[c])
                if not run_proj:
                    continue
                for p in range(NP):
                    ps = pjp.tile([128, QB], F32, tag=f"pt{p % 2}",
                                  name=f"q{c}{p}")
                    for k in range(NKT):
                        nc.tensor.matmul(
                            ps, _r(wq_sb[:, k, p * 128:(p + 1) * 128]),
                            _r(xs[:, k, :]),
                            start=(k == 0), stop=(k == NKT - 1))
                    nc.vector.tensor_copy(qt_sb[:, p, csl], ps)
                for p in range(NP):
                    ps = pjp.tile([128, QB], F32, tag=f"pt{2 + p % 2}",
                                  name=f"k{c}{p}")
                    for k in range(NKT):
                        nc.tensor.matmul(
                            ps, _r(wk_sb[:, k, p * 128:(p + 1) * 128]),
                            _r(xs[:, k, :]),
                            start=(k == 0), stop=(k == NKT - 1))
                    nc.scalar.copy(kt_sb[:, p, csl], ps)
                for i in range(4):
                    ps = pjp.tile([128, QB], F32, tag=f"pt{4 + i % 4}",
                                  name=f"v{c}{i}")
                    for k in range(NKT):
                        nc.tensor.matmul(
                            ps, _r(xs[:, k, i * 128:(i + 1) * 128]),
                            _r(wv_sb[:, k, :]),
                            start=(k == 0), stop=(k == NKT - 1))
                    nc.vector.tensor_copy(
                        v_sb[:, c * 4 + i, :, 0:HD],
                        ps.rearrange("p (h e) -> p h e", h=NH))

        if phases == "dma":
            with tc.tile_pool(name="dma_s", bufs=2) as dms:
                for t in range(NST):
                    for nh_ in range(2):
                        ysb = dms.tile([128, 512], F32R, tag="ysb")
                        nc.vector.tensor_copy(
                            ysb, wot_sb[:, nh_, (t % 2) * 512:(t % 2 + 1) * 512])
                        nc.sync.dma_start(
                            y[t * 128:(t + 1) * 128,
                              nh_ * 512:(nh_ + 1) * 512].bitcast(F32R), ysb)
            return

        if phases in ("vqk", "qk", "v"):
            # truncated build for HW bisection: write qt/kt straight out
            with tc.tile_pool(name="tr_s", bufs=2) as trs:
                for c in range(NQB):
                    tr = trs.tile([128, QB], F32R, tag="tr")
                    if phases == "v":
                        nc.vector.tensor_copy(
                            tr, v_sb[:, c, :, :].rearrange(
                                "p h e -> p (h e)")[:, 0:QB])
                    else:
                        nc.vector.tensor_mul(
                            tr, qt_sb[:, 0, c * QB:(c + 1) * QB],
                            kt_sb[:, 0, c * QB:(c + 1) * QB])
                    nc.sync.dma_start(
                        y[c * 128:(c + 1) * 128, 0:QB].bitcast(F32R), tr)
            return

        # ---- attention, head pairs packed on PE row groups ---------------
        with (
            tc.tile_pool(name="at_c", bufs=1) as atc,
            tc.tile_pool(name="at_e", bufs=3) as ate,
            tc.tile_pool(name="at_r", bufs=2) as atr,
            tc.tile_pool(name="wo_s", bufs=2) as wos,
            tc.tile_pool(name="at_pp", bufs=2, space="PSUM") as atpp,
            tc.tile_pool(name="at_po", bufs=1, space="PSUM") as atpo,
            tc.tile_pool(name="wo_p", bufs=2, space="PSUM") as wop,
        ):
            concat_sb = atc.tile([128, NP, S], F32R)
            for qb in range(NQB):
                nkc = (qb + 1) * ND
                qsl = slice(qb * QB, (qb + 1) * QB)
                for p in range(NP):
                    hA, hB = 2 * p, 2 * p + 1
                    psoA = atpo.tile([HD + 1, QB], F32, tag="psoA")
                    psoB = atpo.tile([HD + 1, QB], F32, tag="psoB")
                    # full (below-diagonal) chunks: both heads of the pair
                    # issue adjacent matmuls on PE row groups 0-63/64-127
                    for kc in range(nkc - ND):
                        ksl = slice(kc * KC, (kc + 1) * KC)
                        pp = atpp.tile([128, 2, QB], F32, tag="pp")
                        nc.tensor.matmul(
                            pp[:, 0, :], _r(kt_sb[0:64, p, ksl]),
                            _r(qt_sb[0:64, p, qsl]), start=True, stop=True)
                        nc.tensor.matmul(
                            pp[:, 1, :], _r(kt_sb[64:128, p, ksl]),
                            _r(qt_sb[64:128, p, qsl]), start=True, stop=True)
                        ex = ate.tile([128, 2, QB], F32R, tag="ex")
                        nc.scalar.activation(ex[:, 0, :], pp[:, 0, :],
                                             AF.Exp, scale=0.125)
                        nc.scalar.activation(ex[:, 1, :], pp[:, 1, :],
                                             AF.Exp, scale=0.125)
                        nc.tensor.matmul(
                            psoA, _r(v_sb[:, kc, hA, :]), _r(ex[:, 0, :]),
                            start=(kc == 0), stop=False)
                        nc.tensor.matmul(
                            psoB, _r(v_sb[:, kc, hB, :]), _r(ex[:, 1, :]),
                            start=(kc == 0), stop=False)
                    # diagonal chunks: exp on the live column subrange,
                    # triangle mask on the partial 128x128 block
                    for j in range(ND):
                        kc = nkc - ND + j
                        ksl = slice(kc * KC, (kc + 1) * KC)
                        q0 = j * KC
                        dsl = slice(qb * QB + q0, (qb + 1) * QB)
                        pp = atpp.tile([128, 2, QB], F32, tag="pp",
                                       name="ppd")
                        nc.tensor.matmul(
                            pp[:, 0, q0:], _r(kt_sb[0:64, p, ksl]),
                            _r(qt_sb[0:64, p, dsl]), start=True, stop=True)
                        nc.tensor.matmul(
                            pp[:, 1, q0:], _r(kt_sb[64:128, p, ksl]),
                            _r(qt_sb[64:128, p, dsl]), start=True, stop=True)
                        ex = ate.tile([128, 2, QB], F32R, tag="ex",
                                      name="exd")
                        nc.scalar.activation(ex[:, 0, q0:], pp[:, 0, q0:],
                                             AF.Exp, scale=0.125)
                        nc.scalar.activation(ex[:, 1, q0:], pp[:, 1, q0:],
                                             AF.Exp, scale=0.125)
                        nc.vector.tensor_mul(ex[:, 0, q0:q0 + KC],
                                             ex[:, 0, q0:q0 + KC], tri_sb)
                        nc.vector.tensor_mul(ex[:, 1, q0:q0 + KC],
                                             ex[:, 1, q0:q0 + KC], tri_sb)
                        nc.tensor.matmul(
                            psoA[:, q0:], _r(v_sb[:, kc, hA, :]),
                            _r(ex[:, 0, q0:]),
                            start=(kc == 0), stop=(kc == nkc - 1))
                        nc.tensor.matmul(
                            psoB[:, q0:], _r(v_sb[:, kc, hB, :]),
                            _r(ex[:, 1, q0:]),
                            start=(kc == 0), stop=(kc == nkc - 1))
                    # normalize: divide rows 0-63 by the denominator row
                    for pso, r0 in ((psoA, 0), (psoB, 64)):
                        recip = atr.tile([1, QB], F32, tag="recip")
                        nc.vector.reciprocal(recip, pso[HD:HD + 1, :])
                        recip_b = atr.tile([64, QB], F32, tag="recip_b")
                        nc.gpsimd.partition_broadcast(recip_b, recip)
                        nc.vector.tensor_mul(
                            concat_sb[r0:r0 + 64, p, qsl], pso[0:HD, :],
                            recip_b)

                # W_O for the q rows finished by this q block
                for t in range(qb * QB // 128, (qb + 1) * QB // 128):
                    ysb = wos.tile([128, D], F32, tag="ysb")
                    for nh_ in range(2):
                        psy = wop.tile([128, 512], F32, tag="psy")
                        for cc in range(CW // 128):
                            nc.tensor.matmul(
                                psy,
                                _r(concat_sb[:, cc, t * 128:(t + 1) * 128]),
                                _r(wot_sb[:, cc, nh_ * 512:(nh_ + 1) * 512]),
                                start=(cc == 0), stop=(cc == CW // 128 - 1))
                        nc.vector.tensor_copy(
                            ysb[:, nh_ * 512:(nh_ + 1) * 512], psy)
                    nc.sync.dma_start(y[t * 128:(t + 1) * 128, :], ysb)


def shard_inputs(x, Wq, Wk, Wv, W_O):
    """Build the 8 per-core input maps from full inputs."""
    masks = (np.arange(KC)[:, None] <= np.arange(KC)[None, :]).astype(
        np.float32)

    def wtile(w):
        # [D, CW] -> [128, NKT, CW] with row d = k*128 + p
        return np.ascontiguousarray(w.reshape(NKT, 128, CW).transpose(1, 0, 2))

    in_maps = []
    for c in range(N_CORES):
        b, g = c // 2, c % 2
        hs = slice(g * NH, (g + 1) * NH)
        xT = np.ascontiguousarray(x[b].T)
        x_ck = np.ascontiguousarray(
            xT.reshape(NKT, 128, NQB, QB).transpose(2, 1, 0, 3))
        wot = np.ascontiguousarray(W_O[:, g * CW:(g + 1) * CW].T)
        in_maps.append({
            "x_ck": x_ck,
            "wq": wtile(Wq[hs].transpose(1, 0, 2).reshape(D, CW)),
            "wk": wtile(Wk[hs].transpose(1, 0, 2).reshape(D, CW)),
            "wv": wtile(Wv[hs].transpose(1, 0, 2).reshape(D, CW)),
            "wot": np.ascontiguousarray(
                wot.reshape(CW // 128, 128, D).transpose(1, 0, 2)),
            "masks": masks,
            "ones": np.ones((128, NST * NH), np.float32),
        })
    return in_maps


def kernel(x, Wq, Wk, Wv, W_O):
    x = np.asarray(x, np.float32)
    Wq = np.asarray(Wq, np.float32)
    Wk = np.asarray(Wk, np.float32)
    Wv = np.asarray(Wv, np.float32)
    W_O = np.asarray(W_O, np.float32)

    if "nc" not in _cache:
        _cache["nc"] = build_nc()
    nc = _cache["nc"]

    in_maps = shard_inputs(x, Wq, Wk, Wv, W_O)
    res = run_bass_kernel_spmd(nc, in_maps, core_ids=list(range(N_CORES)))
    _cache["last_results"] = res

    y = np.zeros((B, S, D), np.float32)
    for c in range(N_CORES):
        y[c // 2] += res.results[c]["y"]
    return y


# revision 4
# speedup vs baseline: 1.0005x; 1.0005x over previous
"""Multi-head causal attention (B=4, S=2048, D=1024, H=16) on 8 NeuronCores.

Sharding: core c -> batch b = c//2, head-group g = c%2 (8 heads each).
Each core computes, for its batch and heads:
    QT/KT = W.T @ x.T          (transposed projections, [64, S] per head)
    V     = x @ Wv             (natural layout, plus ones column for denom)
    ST    = K_chunk @ Q_blk.T  ([k=128, q<=1024] score chunks, causal-skipped)
    E     = exp(ST/8) (* triangle mask on the partial diagonal block)
    accT  = V_aug.T @ E        ([65, q]: rows 0-63 unnormalized out.T, row 64 denom)
    out.T = accT[:64] / accT[64]  (stacked over heads -> concatT [512, S])
    yT_part = W_O_part @ concatT  (output produced transposed, [D, S])
Host transposes and sums the two partial y's per batch.

v3 notes: fp32r matmuls self-load their stationary operand (~150 ns serial
per fresh lhsT), so the loop structure maximizes consecutive same-lhsT
matmuls: attention processes 1024 q columns per head-chunk (two N=512
matmuls sharing one K/V load), projections share each W k-tile across a
pair of x chunks, and W_O runs transposed so each wot tile is shared by
two s-block matmuls.  Exps cover 1024 PSUM columns per op (the per-op
PSUM-read penalty is ~280 ns, so bigger ops win).
"""

import numpy as np

import concourse.bass as bass
import concourse.tile as tile
import concourse.mybir as mybir
from concourse import bacc
from concourse.bass_utils import run_bass_kernel_spmd

B, S, D, H, HD = 4, 2048, 1024, 16, 64
NH = 8            # heads per core
NP = NH // 2      # head pairs per core
QB = 512          # matmul moving-operand block (fp32 N max)
QW = 1024         # attention q super-block (2 matmuls per weight load)
NQW = S // QW     # 2
NQB = S // QB     # 4
KC = 128          # k chunk size
NDW = QW // KC    # diagonal chunks per q super-block (8)
NKT = D // 128    # 8 contraction tiles over D
NST = S // 128    # 16 s tiles
CW = NH * HD      # 512 concat width per core

F32 = mybir.dt.float32
F32R = mybir.dt.float32r
AF = mybir.ActivationFunctionType

N_CORES = 8

_cache = {}


def _r(ap):
    return ap.bitcast(F32R)


def build_nc(repeats=1, phases="full", hw_loop=False):
    nc = bacc.Bacc("TRN2", target_bir_lowering=False, debug=False,
                   num_devices=N_CORES)
    x_ck = nc.dram_tensor("x_ck", [NQB, 128, NKT, QB], F32R,
                          kind="ExternalInput").ap()
    wq = nc.dram_tensor("wq", [128, NKT, CW], F32R, kind="ExternalInput").ap()
    wk = nc.dram_tensor("wk", [128, NKT, CW], F32R, kind="ExternalInput").ap()
    wv = nc.dram_tensor("wv", [128, NKT, CW], F32R, kind="ExternalInput").ap()
    wot = nc.dram_tensor("wot", [128, CW // 128, D], F32R,
                         kind="ExternalInput").ap()
    masks = nc.dram_tensor("masks", [KC, KC], F32R, kind="ExternalInput").ap()
    ones = nc.dram_tensor("ones", [128, NST * NH], F32R,
                          kind="ExternalInput").ap()
    yt = nc.dram_tensor("yt", [D, S], F32, kind="ExternalOutput").ap()

    with tile.TileContext(nc) as tc:
        if hw_loop:
            with tc.For_i(0, repeats, 1):
                _build(tc, x_ck, wq, wk, wv, wot, masks, ones, yt, phases)
        else:
            for _ in range(repeats):
                _build(tc, x_ck, wq, wk, wv, wot, masks, ones, yt, phases)
    nc.compile()
    return nc


def _build(tc, x_ck, wq, wk, wv, wot, masks, ones, yt, phases="full"):
    nc = tc.nc
    with tc.tile_pool(name="persist", bufs=1) as persist:
        qt_sb = persist.tile([128, NP, S], F32R)      # [2 heads, pair, s]
        kt_sb = persist.tile([128, NP, S], F32R)
        v_sb = persist.tile([128, NST, NH, HD + 1], F32R)
        tri_sb = persist.tile([128, KC], F32R)
        wot_sb = persist.tile([128, CW // 128, D], F32R)
        nc.sync.dma_start(tri_sb, masks)
        v_ones = bass.AP(tensor=v_sb.tensor, offset=v_sb.offset + HD,
                         ap=[list(v_sb.ap[0]), [HD + 1, NST * NH], [1, 1]])
        nc.sync.dma_start(v_ones, ones.rearrange("p (n o) -> p n o", o=1))

        # ---- projections: one streamed pass; each W k-tile loaded once per
        # ---- chunk pair (the two chunks' matmuls share the stationary)
        run_proj = phases != "dma"
        with (
            tc.tile_pool(name="pj_w", bufs=1) as pjw,
            tc.tile_pool(name="pj_x", bufs=1) as pjx,
            tc.tile_pool(name="pj_p", bufs=1, space="PSUM") as pjp,
        ):
            wq_sb = pjw.tile([128, NKT, CW], F32R)
            wk_sb = pjw.tile([128, NKT, CW], F32R)
            wv_sb = pjw.tile([128, NKT, CW], F32R)
            # weight DMAs on the ACT HWDGE ring; x chunks ride the SP ring
            nc.scalar.dma_start(wq_sb, wq)
            nc.scalar.dma_start(wk_sb, wk)
            nc.scalar.dma_start(wv_sb, wv)
            nc.scalar.dma_start(wot_sb, wot)
            ti = 0
            for cp in range(2):
                c0, c1 = 2 * cp, 2 * cp + 1
                xs0 = pjx.tile([128, NKT, QB], F32R, tag="xs0")
                xs1 = pjx.tile([128, NKT, QB], F32R, tag="xs1")
                nc.sync.dma_start(xs0, x_ck[c0])
                nc.sync.dma_start(xs1, x_ck[c1])
                if not run_proj:
                    continue
                sl0 = slice(c0 * QB, (c0 + 1) * QB)
                sl1 = slice(c1 * QB, (c1 + 1) * QB)
                for wsb, dst, drain in ((wq_sb, qt_sb, "dve"),
                                        (wk_sb, kt_sb, "act")):
                    for p in range(NP):
                        t0 = pjp.tile([128, QB], F32, tag=f"t{ti % 8}")
                        t1 = pjp.tile([128, QB], F32, tag=f"t{(ti + 1) % 8}")
                        ti += 2
                        for k in range(NKT):
                            lhs = _r(wsb[:, k, p * 128:(p + 1) * 128])
                            nc.tensor.matmul(t0, lhs, _r(xs0[:, k, :]),
                                             start=(k == 0),
                                             stop=(k == NKT - 1))
                            nc.tensor.matmul(t1, lhs, _r(xs1[:, k, :]),
                                             start=(k == 0),
                                             stop=(k == NKT - 1))
                        if drain == "dve":
                            nc.vector.tensor_copy(dst[:, p, sl0], t0)
                            nc.vector.tensor_copy(dst[:, p, sl1], t1)
                        else:
                            nc.scalar.copy(dst[:, p, sl0], t0)
                            nc.scalar.copy(dst[:, p, sl1], t1)
                for ci, xs in ((c0, xs0), (c1, xs1)):
                    for i in range(4):
                        tv = pjp.tile([128, QB], F32, tag=f"t{ti % 8}")
                        ti += 1
                        for k in range(NKT):
                            nc.tensor.matmul(
                                tv, _r(xs[:, k, i * 128:(i + 1) * 128]),
                                _r(wv_sb[:, k, :]),
                                start=(k == 0), stop=(k == NKT - 1))
                        nc.vector.tensor_copy(
                            v_sb[:, ci * 4 + i, :, 0:HD],
                            tv.rearrange("p (h e) -> p h e", h=NH))

        if phases == "dma":
            with tc.tile_pool(name="dma_s", bufs=2) as dms:
                for t in range(8):
                    for half in range(2):
                        ysb = dms.tile([128, 1024], F32R, tag="ysb")
                        nc.vector.tensor_copy(
                            ysb, wot_sb[:, t % 4, :].rearrange("p d -> p d"))
                        nc.sync.dma_start(
                            yt[t * 128:(t + 1) * 128,
                               half * 1024:(half + 1) * 1024].bitcast(F32R),
                            ysb)
            return

        if phases in ("vqk", "qk", "v"):
            # truncated build for HW bisection: write qt/kt straight out
            with tc.tile_pool(name="tr_s", bufs=2) as trs:
                for c in range(NQB):
                    tr = trs.tile([128, QB], F32R, tag="tr")
                    if phases == "v":
                        nc.vector.tensor_copy(
                            tr, v_sb[:, c, :, :].rearrange(
                                "p h e -> p (h e)")[:, 0:QB])
                    else:
                        nc.vector.tensor_mul(
                            tr, qt_sb[:, 0, c * QB:(c + 1) * QB],
                            kt_sb[:, 0, c * QB:(c + 1) * QB])
                    nc.sync.dma_start(
                        yt[c * 128:(c + 1) * 128, 0:QB].bitcast(F32R), tr)
            return

        # ---- attention: 1024 q per head-chunk, shared K/V weight loads ----
        with (
            tc.tile_pool(name="at_c", bufs=1) as atc,
            tc.tile_pool(name="at_e", bufs=3) as ate,
            tc.tile_pool(name="at_r", bufs=2) as atr,
            tc.tile_pool(name="wo_s", bufs=2) as wos,
            tc.tile_pool(name="at_pp", bufs=2, space="PSUM") as atpp,
            tc.tile_pool(name="at_po", bufs=1, space="PSUM") as atpo,
            tc.tile_pool(name="wo_p", bufs=1, space="PSUM") as wop,
        ):
            concat_sb = atc.tile([128, NP, S], F32R)
            for qw in range(NQW):
                w0 = qw * QW
                nfull = qw * NDW            # full chunks below this block
                for h in range(NH):
                    p, r0 = h // 2, 64 * (h % 2)
                    pso = atpo.tile([HD + 1, 2, QB], F32, tag="pso")
                    pso_f = pso.rearrange("p a b -> p (a b)")
                    for kc in range(nfull):
                        ksl = slice(kc * KC, (kc + 1) * KC)
                        pp = atpp.tile([128, 2, QB], F32, tag="pp")
                        lhs = _r(kt_sb[r0:r0 + 64, p, ksl])
                        nc.tensor.matmul(
                            pp[:, 0, :], lhs,
                            _r(qt_sb[r0:r0 + 64, p, w0:w0 + QB]),
                            start=True, stop=True)
                        nc.tensor.matmul(
                            pp[:, 1, :], lhs,
                            _r(qt_sb[r0:r0 + 64, p, w0 + QB:w0 + QW]),
                            start=True, stop=True)
                        ex = ate.tile([128, 2, QB], F32R, tag="ex")
                        nc.scalar.activation(ex, pp, AF.Exp, scale=0.125)
                        vlhs = _r(v_sb[:, kc, h, :])
                        nc.tensor.matmul(pso[:, 0, :], vlhs, _r(ex[:, 0, :]),
                                         start=(kc == 0), stop=False)
                        nc.tensor.matmul(pso[:, 1, :], vlhs, _r(ex[:, 1, :]),
                                         start=(kc == 0), stop=False)
                    # diagonal chunks: scores only on live columns [q0:QW)
                    for j in range(NDW):
                        kc = nfull + j
                        ksl = slice(kc * KC, (kc + 1) * KC)
                        q0 = j * KC
                        pp = atpp.tile([128, 2, QB], F32, tag="pp",
                                       name="ppd")
                        pp_f = pp.rearrange("p a b -> p (a b)")
                        lhs = _r(kt_sb[r0:r0 + 64, p, ksl])
                        if q0 < QB:
                            nc.tensor.matmul(
                                pp_f[:, q0:QB], lhs,
                                _r(qt_sb[r0:r0 + 64, p, w0 + q0:w0 + QB]),
                                start=True, stop=True)
                            nc.tensor.matmul(
                                pp_f[:, QB:], lhs,
                                _r(qt_sb[r0:r0 + 64, p, w0 + QB:w0 + QW]),
                                start=True, stop=True)
                        else:
                            nc.tensor.matmul(
                                pp_f[:, q0:], lhs,
                                _r(qt_sb[r0:r0 + 64, p, w0 + q0:w0 + QW]),
                                start=True, stop=True)
                        ex = ate.tile([128, 2, QB], F32R, tag="ex",
                                      name="exd")
                        ex_f = ex.rearrange("p a b -> p (a b)")
                        nc.scalar.activation(ex_f[:, q0:], pp_f[:, q0:],
                                             AF.Exp, scale=0.125)
                        nc.vector.tensor_mul(ex_f[:, q0:q0 + KC],
                                             ex_f[:, q0:q0 + KC], tri_sb)
                        vlhs = _r(v_sb[:, kc, h, :])
                        stop0 = (j == (QB // KC) - 1)
                        if q0 < QB:
                            nc.tensor.matmul(
                                pso[:, 0, q0:], vlhs, _r(ex[:, 0, q0:]),
                                start=(kc == 0), stop=stop0)
                            nc.tensor.matmul(
                                pso[:, 1, :], vlhs, _r(ex[:, 1, :]),
                                start=(kc == 0), stop=(j == NDW - 1))
                        else:
                            nc.tensor.matmul(
                                pso[:, 1, q0 - QB:], vlhs,
                                _r(ex[:, 1, q0 - QB:]),
                                start=False, stop=(j == NDW - 1))
                    # normalize: divide rows 0-63 by the denominator row
                    recip = atr.tile([1, QW], F32, tag="recip")
                    nc.vector.reciprocal(recip, pso_f[HD:HD + 1, :])
                    recip_b = atr.tile([64, QW], F32, tag="recip_b")
                    nc.gpsimd.partition_broadcast(recip_b, recip)
                    nc.vector.tensor_mul(
                        concat_sb[r0:r0 + 64, p, w0:w0 + QW],
                        pso_f[0:HD, :], recip_b)

                # W_O transposed: yT[d, s] = sum_c wot[c, d] * concat[c, s];
                # each wot tile is stationary for two N=512 matmuls
                for dt_ in range(D // 128):
                    dsl = slice(dt_ * 128, (dt_ + 1) * 128)
                    ysb = wos.tile([128, QW], F32, tag="ysb")
                    psy = wop.tile([128, 2, QB], F32, tag="psy")
                    for cc in range(CW // 128):
                        lhs = _r(wot_sb[:, cc, dsl])
                        nc.tensor.matmul(
                            psy[:, 0, :], lhs,
                            _r(concat_sb[:, cc, w0:w0 + QB]),
                            start=(cc == 0), stop=(cc == CW // 128 - 1))
                        nc.tensor.matmul(
                            psy[:, 1, :], lhs,
                            _r(concat_sb[:, cc, w0 + QB:w0 + QW]),
                            start=(cc == 0), stop=(cc == CW // 128 - 1))
                    nc.vector.tensor_copy(
                        ysb, psy.rearrange("p a b -> p (a b)"))
                    nc.sync.dma_start(yt[dsl, w0:w0 + QW], ysb)


def shard_inputs(x, Wq, Wk, Wv, W_O):
    """Build the 8 per-core input maps from full inputs."""
    masks = (np.arange(KC)[:, None] <= np.arange(KC)[None, :]).astype(
        np.float32)

    def wtile(w):
        # [D, CW] -> [128, NKT, CW] with row d = k*128 + p
        return np.ascontiguousarray(w.reshape(NKT, 128, CW).transpose(1, 0, 2))

    in_maps = []
    for c in range(N_CORES):
        b, g = c // 2, c % 2
        hs = slice(g * NH, (g + 1) * NH)
        xT = np.ascontiguousarray(x[b].T)
        x_ck = np.ascontiguousarray(
            xT.reshape(NKT, 128, NQB, QB).transpose(2, 1, 0, 3))
        wot = np.ascontiguousarray(W_O[:, g * CW:(g + 1) * CW].T)
        in_maps.append({
            "x_ck": x_ck,
            "wq": wtile(Wq[hs].transpose(1, 0, 2).reshape(D, CW)),
            "wk": wtile(Wk[hs].transpose(1, 0, 2).reshape(D, CW)),
            "wv": wtile(Wv[hs].transpose(1, 0, 2).reshape(D, CW)),
            "wot": np.ascontiguousarray(
                wot.reshape(CW // 128, 128, D).transpose(1, 0, 2)),
            "masks": masks,
            "ones": np.ones((128, NST * NH), np.float32),
        })
    return in_maps


def kernel(x, Wq, Wk, Wv, W_O):
    x = np.asarray(x, np.float32)
    Wq = np.asarray(Wq, np.float32)
    Wk = np.asarray(Wk, np.float32)
    Wv = np.asarray(Wv, np.float32)
    W_O = np.asarray(W_O, np.float32)

    if "nc" not in _cache:
        _cache["nc"] = build_nc()
    nc = _cache["nc"]

    in_maps = shard_inputs(x, Wq, Wk, Wv, W_O)
    res = run_bass_kernel_spmd(nc, in_maps, core_ids=list(range(N_CORES)))
    _cache["last_results"] = res

    y = np.zeros((B, S, D), np.float32)
    for c in range(N_CORES):
        y[c // 2] += res.results[c]["yt"].T
    return y
